# revision 14
# baseline (speedup 1.0000x reference)
"""Mixtral sparse MoE block on 8 Trainium2 NeuronCores.

Strategy (expert-parallel, per the problem's sharding hint):
  * Host computes the router (tiny matmul + softmax + top-2) and dispatches
    tokens to experts: core e receives the tokens routed to expert e
    (gathered + transposed to [H, C]), that expert's w1/w3/w2 slices, and the
    per-token combine weights.
  * Each core runs a fused gated-FFN (silu(x@w1) * (x@w3)) @ w2 over its
    gathered tokens, scaling by the combine weight on-device.
  * Host scatter-adds the per-expert outputs back into the [T, H] output.

All matmuls run as float32r (fp32 data at full PE rate). Tokens ride the
moving (free) axis: h1^T = w1^T·x^T etc., so w1/w3/w2 natural layouts serve
directly as stationary operands and no transposes are needed on-device.
Each weight pass processes NP token chunks so expert weights are streamed
from HBM only ceil(C/(NP*TC)) times; weight DMAs alternate between the two
hardware DGE queues (sync + scalar engines).
"""

import os
import sys

for _p in ("/root/.axon_site/_ro/trn_rl_repo", "/opt/trn_rl_repo"):
    if _p not in sys.path and os.path.isdir(_p):
        sys.path.append(_p)

import numpy as np

import concourse.bass as bass
import concourse.mybir as mybir
from concourse import bacc
from concourse.bass_utils import run_bass_kernel_spmd
from concourse.tile import TileContext

HID, FFN, E, TOPK = 2048, 4096, 8, 2
P = 128
HB = HID // P  # 16 h-tiles
FB = FFN // P  # 32 f-tiles
TC = 352  # token chunk (PSUM tile width)
NP = 2  # token chunks per weight pass

MM_MODE = os.environ.get("MOE_MM_MODE", "f32r")  # f32r | f32 | bf16

_build_cache = {}


def _dt_store():
    if MM_MODE == "bf16":
        return mybir.dt.bfloat16
    if MM_MODE == "f32r":
        return mybir.dt.float32r
    return mybir.dt.float32


def _np_store():
    if MM_MODE == "bf16":
        import ml_dtypes

        return ml_dtypes.bfloat16
    return np.float32


def build_moe_nc(C):
    """One expert's gated FFN over C gathered tokens (tokens on free axis)."""
    GRP = NP * TC
    assert C % GRP == 0
    npasses = C // GRP
    dts = _dt_store()
    f32 = mybir.dt.float32

    nc = bacc.Bacc()
    xt = nc.dram_tensor("xt", [HID, C], dts, kind="ExternalInput")
    w1 = nc.dram_tensor("w1", [HID, FFN], dts, kind="ExternalInput")
    w3 = nc.dram_tensor("w3", [HID, FFN], dts, kind="ExternalInput")
    w2 = nc.dram_tensor("w2", [FFN, HID], dts, kind="ExternalInput")
    coeff = nc.dram_tensor("coeff", [1, C], f32, kind="ExternalInput")
    yt = nc.dram_tensor("yt", [HID, C], f32, kind="ExternalOutput")

    xt_r = xt.rearrange("(hh p) t -> p hh t", p=P)
    yt_r = yt.rearrange("(hh p) t -> p hh t", p=P)
    w1_r = w1.rearrange("(hh p) f -> p hh f", p=P)
    w3_r = w3.rearrange("(hh p) f -> p hh f", p=P)
    w2_r = w2.rearrange("(ff p) h -> p ff h", p=P)

    # alternate weight DMAs across the two HWDGE queues
    dmae = [nc.sync, nc.scalar]

    with TileContext(nc) as tc:
        with (
            tc.tile_pool(name="xp", bufs=1) as xp,
            tc.tile_pool(name="wp", bufs=1) as wp,
            tc.tile_pool(name="ap_", bufs=1) as ap_,
            tc.tile_pool(name="yp", bufs=1) as yp,
            tc.tile_pool(name="cp", bufs=1) as cp,
            tc.tile_pool(name="ps", bufs=1, space="PSUM") as ps,
        ):
            def load_pass_inputs(g):
                xtiles = []
                cbs = []
                for k in range(NP):
                    tok = slice((g * NP + k) * TC, (g * NP + k + 1) * TC)
                    xtile = xp.tile([P, HB, TC], dts, tag=f"x{k}", name=f"x{k}", bufs=1)
                    nc.sync.dma_start(xtile[:], xt_r[:, :, tok])
                    cb = cp.tile([P, TC], f32, tag=f"cb{k}", name=f"cb{k}", bufs=2)
                    nc.scalar.dma_start(cb[:], coeff[0:1, tok].to_broadcast((P, TC)))
                    xtiles.append(xtile)
                    cbs.append(cb)
                return xtiles, cbs

            nxt = load_pass_inputs(0)
            for g in range(npasses):
                xtiles, cbs = nxt

                # ---- stage 1: A[f, tok] = silu(w1^T x^T) * (w3^T x^T) ----
                atiles = [
                    [
                        ap_.tile([P, TC], dts, tag=f"a{k}_{f}", name=f"a{k}_{f}", bufs=1)
                        for f in range(FB)
                    ]
                    for k in range(NP)
                ]
                for fb in range(FB // 2):  # pairs of f-tiles (256 f-cols)
                    fcols = slice(fb * 2 * P, (fb + 1) * 2 * P)
                    ps1 = [
                        [ps.tile([P, TC], f32, tag="ps", name=f"ps1_{fb}_{i}_{k}", bufs=8) for i in range(2)]
                        for k in range(NP)
                    ]
                    ps3 = [
                        [ps.tile([P, TC], f32, tag="ps", name=f"ps3_{fb}_{i}_{k}", bufs=8) for i in range(2)]
                        for k in range(NP)
                    ]
                    # batched weight loads: [128, 8 h-tiles, 256 f-cols] = 1 MB per DMA
                    for hq in range(2):
                        hhs = slice(hq * (HB // 2), (hq + 1) * (HB // 2))
                        w1t = wp.tile([P, HB // 2, 2 * P], dts, tag="w1", name="w1t", bufs=2)
                        w3t = wp.tile([P, HB // 2, 2 * P], dts, tag="w3", name="w3t", bufs=2)
                        dmae[0].dma_start(w1t[:], w1_r[:, hhs, fcols])
                        dmae[1].dma_start(w3t[:], w3_r[:, hhs, fcols])
                        for hl in range(HB // 2):
                            hh = hq * (HB // 2) + hl
                            st = dict(start=(hh == 0), stop=(hh == HB - 1))
                            for i in range(2):
                                for k in range(NP):
                                    nc.tensor.matmul(
                                        ps1[k][i][:], w1t[:, hl, i * P : (i + 1) * P],
                                        xtiles[k][:, hh, :], **st,
                                    )
                                for k in range(NP):
                                    nc.tensor.matmul(
                                        ps3[k][i][:], w3t[:, hl, i * P : (i + 1) * P],
                                        xtiles[k][:, hh, :], **st,
                                    )
                    for i in range(2):
                        f = fb * 2 + i
                        for k in range(NP):
                            s1 = yp.tile([P, TC], f32, tag="s1", name="s1", bufs=4)
                            nc.scalar.activation(
                                s1[:], ps1[k][i][:], mybir.ActivationFunctionType.Silu
                            )
                            nc.vector.tensor_mul(atiles[k][f][:], s1[:], ps3[k][i][:])

                # queue next pass's activations ahead of stage-2 weight DMAs:
                # both HWDGE queues are FIFO, so emitting here lets the x
                # transfer overlap stage 2 instead of trailing its w2 loads.
                if g + 1 < npasses:
                    nxt = load_pass_inputs(g + 1)

                # ---- stage 2: y^T[h, tok] = w2^T A ----
                for hb in range(HB // 2):  # pairs of h-tiles (256 h-cols)
                    hcols = slice(hb * 2 * P, (hb + 1) * 2 * P)
                    psy = [
                        [ps.tile([P, TC], f32, tag="ps", name=f"psy_{hb}_{i}_{k}", bufs=8) for i in range(2)]
                        for k in range(NP)
                    ]
                    for fq in range(4):
                        ffs = slice(fq * (FB // 4), (fq + 1) * (FB // 4))
                        w2t = wp.tile([P, FB // 4, 2 * P], dts, tag="w2", name="w2t", bufs=2)
                        dmae[fq % 2].dma_start(w2t[:], w2_r[:, ffs, hcols])
                        for fl in range(FB // 4):
                            f = fq * (FB // 4) + fl
                            st = dict(start=(f == 0), stop=(f == FB - 1))
                            for i in range(2):
                                for k in range(NP):
                                    nc.tensor.matmul(
                                        psy[k][i][:], w2t[:, fl, i * P : (i + 1) * P],
                                        atiles[k][f][:], **st,
                                    )
                    for i in range(2):
                        hh = hb * 2 + i
                        for k in range(NP):
                            tok = slice((g * NP + k) * TC, (g * NP + k + 1) * TC)
                            yt_t = yp.tile([P, TC], f32, tag="y", name="yt_t", bufs=4)
                            nc.vector.tensor_mul(yt_t[:], psy[k][i][:], cbs[k][:])
                            dmae[i].dma_start(yt_r[:, hh, tok], yt_t[:])

    nc.finalize()
    return nc


def _get_nc(C):
    key = (C, MM_MODE)
    if key not in _build_cache:
        _build_cache[key] = build_moe_nc(C)
    return _build_cache[key]


def _route(x, gate_w):
    """Replicates the reference router in numpy fp32."""
    logits = x @ gate_w  # [T, E] fp32
    m = logits.max(axis=-1, keepdims=True)
    p = np.exp(logits - m)
    p /= p.sum(axis=-1, keepdims=True)
    sel = np.argsort(-p, axis=-1, kind="stable")[:, :TOPK]  # [T, K]
    topw = np.take_along_axis(p, sel, axis=-1)
    topw = topw / topw.sum(axis=-1, keepdims=True)
    return logits, sel, topw.astype(np.float32)


def kernel(hidden_states, gate_w, w1, w3, w2, _trace=False):
    x = np.ascontiguousarray(np.asarray(hidden_states, dtype=np.float32)).reshape(
        -1, HID
    )
    gate_w = np.asarray(gate_w, dtype=np.float32)
    w1 = np.asarray(w1, dtype=np.float32)
    w3 = np.asarray(w3, dtype=np.float32)
    w2 = np.asarray(w2, dtype=np.float32)
    T = x.shape[0]

    logits, sel, topw = _route(x, gate_w)

    # dispatch: token lists per expert
    idx = [None] * E
    cof = [None] * E
    flat_sel = sel.ravel()
    flat_tok = np.repeat(np.arange(T), TOPK)
    flat_w = topw.ravel()
    order = np.argsort(flat_sel, kind="stable")
    bounds = np.searchsorted(flat_sel[order], np.arange(E + 1))
    for e in range(E):
        seg = order[bounds[e] : bounds[e + 1]]
        idx[e] = flat_tok[seg]
        cof[e] = flat_w[seg]
    maxn = max(len(i) for i in idx)
    GRP = NP * TC
    C = ((maxn + GRP - 1) // GRP) * GRP

    nc = _get_nc(C)

    nps = _np_store()
    xT = np.ascontiguousarray(x.T)  # [H, T]
    in_maps = []
    for e in range(E):
        n = len(idx[e])
        xt_e = np.zeros((HID, C), dtype=nps)
        xt_e[:, :n] = xT[:, idx[e]]
        co = np.zeros((1, C), dtype=np.float32)
        co[0, :n] = cof[e]
        in_maps.append(
            {
                "xt": xt_e,
                "w1": np.ascontiguousarray(w1[e]).astype(nps, copy=False),
                "w3": np.ascontiguousarray(w3[e]).astype(nps, copy=False),
                "w2": np.ascontiguousarray(w2[e]).astype(nps, copy=False),
                "coeff": co,
            }
        )

    res = run_bass_kernel_spmd(nc, in_maps, list(range(E)), trace=_trace)
    kernel.last_results = res

    out = np.zeros((T, HID), dtype=np.float32)
    for e in range(E):
        n = len(idx[e])
        if n:
            out[idx[e]] += res.results[e]["yt"][:, :n].T
    return out.reshape(hidden_states.shape), logits


# revision 15
# speedup vs baseline: 1.0271x; 1.0271x over previous
"""Mixtral sparse MoE block on 8 Trainium2 NeuronCores.

Strategy (expert-parallel, per the problem's sharding hint):
  * Host computes the router (tiny matmul + softmax + top-2) and dispatches
    tokens to experts: core e receives the tokens routed to expert e
    (gathered + transposed to [H, C]), that expert's w1/w3/w2 slices, and the
    per-token combine weights.
  * Each core runs a fused gated-FFN (silu(x@w1) * (x@w3)) @ w2 over its
    gathered tokens, scaling by the combine weight on-device.
  * Host scatter-adds the per-expert outputs back into the [T, H] output.

All matmuls run as float32r (fp32 data at full PE rate). Tokens ride the
moving (free) axis: h1^T = w1^T·x^T etc., so w1/w3/w2 natural layouts serve
directly as stationary operands and no transposes are needed on-device.
Each weight pass processes NP token chunks so expert weights are streamed
from HBM only ceil(C/(NP*TC)) times; weight DMAs alternate between the two
hardware DGE queues (sync + scalar engines).
"""

import os
import sys

for _p in ("/root/.axon_site/_ro/trn_rl_repo", "/opt/trn_rl_repo"):
    if _p not in sys.path and os.path.isdir(_p):
        sys.path.append(_p)

import numpy as np

import concourse.bass as bass
import concourse.mybir as mybir
from concourse import bacc
from concourse.bass_utils import run_bass_kernel_spmd
from concourse.tile import TileContext

HID, FFN, E, TOPK = 2048, 4096, 8, 2
P = 128
HB = HID // P  # 16 h-tiles
FB = FFN // P  # 32 f-tiles
TC = 352  # token chunk (PSUM tile width)
NP = 2  # token chunks per weight pass

MM_MODE = os.environ.get("MOE_MM_MODE", "f32r")  # f32r | f32 | bf16

_build_cache = {}


def _dt_store():
    if MM_MODE == "bf16":
        return mybir.dt.bfloat16
    if MM_MODE == "f32r":
        return mybir.dt.float32r
    return mybir.dt.float32


def _np_store():
    if MM_MODE == "bf16":
        import ml_dtypes

        return ml_dtypes.bfloat16
    return np.float32


def build_moe_nc(C):
    """One expert's gated FFN over C gathered tokens (tokens on free axis)."""
    GRP = NP * TC
    assert C % GRP == 0
    npasses = C // GRP
    dts = _dt_store()
    f32 = mybir.dt.float32

    nc = bacc.Bacc()
    xt = nc.dram_tensor("xt", [HID, C], dts, kind="ExternalInput")
    w1 = nc.dram_tensor("w1", [HID, FFN], dts, kind="ExternalInput")
    w3 = nc.dram_tensor("w3", [HID, FFN], dts, kind="ExternalInput")
    w2 = nc.dram_tensor("w2", [FFN, HID], dts, kind="ExternalInput")
    coeff = nc.dram_tensor("coeff", [1, C], f32, kind="ExternalInput")
    yt = nc.dram_tensor("yt", [HID, C], f32, kind="ExternalOutput")

    xt_r = xt.rearrange("(hh p) t -> p hh t", p=P)
    yt_r = yt.rearrange("(hh p) t -> p hh t", p=P)
    w1_r = w1.rearrange("(hh p) f -> p hh f", p=P)
    w3_r = w3.rearrange("(hh p) f -> p hh f", p=P)
    w2_r = w2.rearrange("(ff p) h -> p ff h", p=P)

    # alternate weight DMAs across the two HWDGE queues
    dmae = [nc.sync, nc.scalar]

    with TileContext(nc) as tc:
        with (
            tc.tile_pool(name="xp", bufs=1) as xp,
            tc.tile_pool(name="wp", bufs=1) as wp,
            tc.tile_pool(name="ap_", bufs=1) as ap_,
            tc.tile_pool(name="yp", bufs=1) as yp,
            tc.tile_pool(name="cp", bufs=1) as cp,
            tc.tile_pool(name="ps", bufs=1, space="PSUM") as ps,
        ):
            def load_pass_inputs(g):
                xtiles = []
                cbs = []
                for k in range(NP):
                    tok = slice((g * NP + k) * TC, (g * NP + k + 1) * TC)
                    xtile = xp.tile([P, HB, TC], dts, tag=f"x{k}", name=f"x{k}", bufs=1)
                    nc.gpsimd.dma_start(xtile[:], xt_r[:, :, tok])
                    cb = cp.tile([P, TC], f32, tag=f"cb{k}", name=f"cb{k}", bufs=2)
                    nc.gpsimd.dma_start(cb[:], coeff[0:1, tok].to_broadcast((P, TC)))
                    xtiles.append(xtile)
                    cbs.append(cb)
                return xtiles, cbs

            nxt = load_pass_inputs(0)
            for g in range(npasses):
                xtiles, cbs = nxt

                # ---- stage 1: A[f, tok] = silu(w1^T x^T) * (w3^T x^T) ----
                atiles = [
                    [
                        ap_.tile([P, TC], dts, tag=f"a{k}_{f}", name=f"a{k}_{f}", bufs=1)
                        for f in range(FB)
                    ]
                    for k in range(NP)
                ]
                for fb in range(FB // 2):  # pairs of f-tiles (256 f-cols)
                    fcols = slice(fb * 2 * P, (fb + 1) * 2 * P)
                    ps1 = [
                        [ps.tile([P, TC], f32, tag="ps", name=f"ps1_{fb}_{i}_{k}", bufs=8) for i in range(2)]
                        for k in range(NP)
                    ]
                    ps3 = [
                        [ps.tile([P, TC], f32, tag="ps", name=f"ps3_{fb}_{i}_{k}", bufs=8) for i in range(2)]
                        for k in range(NP)
                    ]
                    # batched weight loads: [128, 8 h-tiles, 256 f-cols] = 1 MB per DMA
                    for hq in range(2):
                        hhs = slice(hq * (HB // 2), (hq + 1) * (HB // 2))
                        w1t = wp.tile([P, HB // 2, 2 * P], dts, tag="w1", name="w1t", bufs=2)
                        w3t = wp.tile([P, HB // 2, 2 * P], dts, tag="w3", name="w3t", bufs=2)
                        dmae[0].dma_start(w1t[:], w1_r[:, hhs, fcols])
                        dmae[1].dma_start(w3t[:], w3_r[:, hhs, fcols])
                        for hl in range(HB // 2):
                            hh = hq * (HB // 2) + hl
                            st = dict(start=(hh == 0), stop=(hh == HB - 1))
                            for i in range(2):
                                for k in range(NP):
                                    nc.tensor.matmul(
                                        ps1[k][i][:], w1t[:, hl, i * P : (i + 1) * P],
                                        xtiles[k][:, hh, :], **st,
                                    )
                                for k in range(NP):
                                    nc.tensor.matmul(
                                        ps3[k][i][:], w3t[:, hl, i * P : (i + 1) * P],
                                        xtiles[k][:, hh, :], **st,
                                    )
                    for i in range(2):
                        f = fb * 2 + i
                        for k in range(NP):
                            s1 = yp.tile([P, TC], f32, tag="s1", name="s1", bufs=4)
                            nc.scalar.activation(
                                s1[:], ps1[k][i][:], mybir.ActivationFunctionType.Silu
                            )
                            nc.vector.tensor_mul(atiles[k][f][:], s1[:], ps3[k][i][:])

                # queue next pass's activations ahead of stage-2 weight DMAs:
                # both HWDGE queues are FIFO, so emitting here lets the x
                # transfer overlap stage 2 instead of trailing its w2 loads.
                if g + 1 < npasses:
                    nxt = load_pass_inputs(g + 1)

                # ---- stage 2: y^T[h, tok] = w2^T A ----
                for hb in range(HB // 2):  # pairs of h-tiles (256 h-cols)
                    hcols = slice(hb * 2 * P, (hb + 1) * 2 * P)
                    psy = [
                        [ps.tile([P, TC], f32, tag="ps", name=f"psy_{hb}_{i}_{k}", bufs=8) for i in range(2)]
                        for k in range(NP)
                    ]
                    for fq in range(4):
                        ffs = slice(fq * (FB // 4), (fq + 1) * (FB // 4))
                        w2t = wp.tile([P, FB // 4, 2 * P], dts, tag="w2", name="w2t", bufs=2)
                        dmae[fq % 2].dma_start(w2t[:], w2_r[:, ffs, hcols])
                        for fl in range(FB // 4):
                            f = fq * (FB // 4) + fl
                            st = dict(start=(f == 0), stop=(f == FB - 1))
                            for i in range(2):
                                for k in range(NP):
                                    nc.tensor.matmul(
                                        psy[k][i][:], w2t[:, fl, i * P : (i + 1) * P],
                                        atiles[k][f][:], **st,
                                    )
                    for i in range(2):
                        hh = hb * 2 + i
                        for k in range(NP):
                            tok = slice((g * NP + k) * TC, (g * NP + k + 1) * TC)
                            yt_t = yp.tile([P, TC], f32, tag="y", name="yt_t", bufs=4)
                            nc.vector.tensor_mul(yt_t[:], psy[k][i][:], cbs[k][:])
                            dmae[i].dma_start(yt_r[:, hh, tok], yt_t[:])

    nc.finalize()
    return nc


def _get_nc(C):
    key = (C, MM_MODE)
    if key not in _build_cache:
        _build_cache[key] = build_moe_nc(C)
    return _build_cache[key]


def _route(x, gate_w):
    """Replicates the reference router in numpy fp32."""
    logits = x @ gate_w  # [T, E] fp32
    m = logits.max(axis=-1, keepdims=True)
    p = np.exp(logits - m)
    p /= p.sum(axis=-1, keepdims=True)
    sel = np.argsort(-p, axis=-1, kind="stable")[:, :TOPK]  # [T, K]
    topw = np.take_along_axis(p, sel, axis=-1)
    topw = topw / topw.sum(axis=-1, keepdims=True)
    return logits, sel, topw.astype(np.float32)


def kernel(hidden_states, gate_w, w1, w3, w2, _trace=False):
    x = np.ascontiguousarray(np.asarray(hidden_states, dtype=np.float32)).reshape(
        -1, HID
    )
    gate_w = np.asarray(gate_w, dtype=np.float32)
    w1 = np.asarray(w1, dtype=np.float32)
    w3 = np.asarray(w3, dtype=np.float32)
    w2 = np.asarray(w2, dtype=np.float32)
    T = x.shape[0]

    logits, sel, topw = _route(x, gate_w)

    # dispatch: token lists per expert
    idx = [None] * E
    cof = [None] * E
    flat_sel = sel.ravel()
    flat_tok = np.repeat(np.arange(T), TOPK)
    flat_w = topw.ravel()
    order = np.argsort(flat_sel, kind="stable")
    bounds = np.searchsorted(flat_sel[order], np.arange(E + 1))
    for e in range(E):
        seg = order[bounds[e] : bounds[e + 1]]
        idx[e] = flat_tok[seg]
        cof[e] = flat_w[seg]
    maxn = max(len(i) for i in idx)
    GRP = NP * TC
    C = ((maxn + GRP - 1) // GRP) * GRP

    nc = _get_nc(C)

    nps = _np_store()
    xT = np.ascontiguousarray(x.T)  # [H, T]
    in_maps = []
    for e in range(E):
        n = len(idx[e])
        xt_e = np.zeros((HID, C), dtype=nps)
        xt_e[:, :n] = xT[:, idx[e]]
        co = np.zeros((1, C), dtype=np.float32)
        co[0, :n] = cof[e]
        in_maps.append(
            {
                "xt": xt_e,
                "w1": np.ascontiguousarray(w1[e]).astype(nps, copy=False),
                "w3": np.ascontiguousarray(w3[e]).astype(nps, copy=False),
                "w2": np.ascontiguousarray(w2[e]).astype(nps, copy=False),
                "coeff": co,
            }
        )

    res = run_bass_kernel_spmd(nc, in_maps, list(range(E)), trace=_trace)
    kernel.last_results = res

    out = np.zeros((T, HID), dtype=np.float32)
    for e in range(E):
        n = len(idx[e])
        if n:
            out[idx[e]] += res.results[e]["yt"][:, :n].T
    return out.reshape(hidden_states.shape), logits
